# revision 19
# baseline (speedup 1.0000x reference)
"""NT-Xent loss (SimCLR) on 8 Trainium2 NeuronCores.

Contract: kernel(z_i, z_j) -> np.float32 scalar loss, matching the
reference NT-Xent (temperature 0.5). Inputs are the full [4096, 128]
fp32 projection batches; sharding happens inside.

Strategy (per core c of 8):
  - rows of the 8192x8192 sim matrix are sharded: core c owns rows
    [c*1024, (c+1)*1024).
  - every core redundantly normalizes + transposes the full z
    (concat of z_i, z_j) into zhatT [128(D), 8192] bf16 on-chip; that is
    far cheaper than communicating it.
  - all SBUF loads use a per-partition-contiguous layout (partition p
    holds rows p*64..p*64+63 of z); this permutes rows/columns of the
    sim matrix, which is irrelevant because every result is summed.
  - row norms are computed in fp32 (scalar_tensor_tensor fused
    square+reduce); 1/sqrt via bit-trick seed + 2 Newton steps on the
    vector engine (keeps ScalarE on a single Exp table set).
  - both normalizations fuse into per-partition vector-engine scales in
    the natural layout (rows live on partitions there): slab rows are
    pre-scaled by 2/||row||, columns by 1/||row||, each fused with the
    fp32->bf16 cast; the PE then only runs plain bf16 transposes and
    bf16 sim matmuls, and the PSUM logits come out fully scaled.
  - exp + row-sum are fused in one ScalarE pass (scale=1) via accum_out
    over 2048-wide PSUM tiles (4 banks), double buffered; prep and main
    PSUM tiles share one pool with emission interleaved to match the
    allocator's in-order slot reuse.
  - the diagonal (masked with -inf in the reference) contributes exactly
    exp(2) to each raw row-sum; it is subtracted before the final log.
  - the final per-row log uses an exponent-split + atanh-series
    polynomial evaluated on the vector engine (the Ln activation table
    is not loadable in this runtime).
  - positives are computed from the raw fp32 slab/partner rows (per-core
    inputs), off the critical path.
  - each core writes [128, 16]: cols 0:8 lse per slab row, 8:16 pos per
    slab row. The host sums (lse - pos) over all cores / 8192.
"""

import os
import sys

if "/opt/trn_rl_repo" not in sys.path:
    sys.path.insert(0, "/opt/trn_rl_repo")

import numpy as np

import concourse.bacc as bacc
import concourse.mybir as mybir
import concourse.tile as tile
from concourse.bass_utils import run_bass_kernel_spmd

B = 4096
D = 128
N = 2 * B  # 8192 rows of the sim matrix
CORES = 8
SLAB = N // CORES  # 1024 rows per core
NT = N // 128  # 64 partition-tiles of z
ST = SLAB // 128  # 8 slab tiles
GROUPS = 8
GT = NT // GROUPS
NB = 4  # main-loop column blocks of 2048
EXP2 = float(np.exp(2.0))
LN2 = float(np.log(2.0))
MAGIC = 0x5F3759DF

f32 = mybir.dt.float32
bf16 = mybir.dt.bfloat16
u32 = mybir.dt.uint32


def build_nc():
    nc = bacc.Bacc("TRN2", target_bir_lowering=False, debug=False, num_devices=CORES)
    z = nc.dram_tensor("z", [N, D], f32, kind="ExternalInput").ap()
    zs = nc.dram_tensor("zs", [SLAB, D], f32, kind="ExternalInput").ap()
    zp = nc.dram_tensor("zp", [SLAB, D], f32, kind="ExternalInput").ap()
    eye = nc.dram_tensor("eye", [128, 128], f32, kind="ExternalInput").ap()
    out = nc.dram_tensor("out", [128, 16], f32, kind="ExternalOutput").ap()

    AF = mybir.ActivationFunctionType
    OP = mybir.AluOpType

    with tile.TileContext(nc) as tc:
        with (
            tc.tile_pool(name="big", bufs=1) as big,
            tc.tile_pool(name="stats", bufs=1) as stats,
            tc.tile_pool(name="work", bufs=3) as work,
            tc.tile_pool(name="mm_ps", bufs=2, space="PSUM") as mm_ps_pool,
        ):
            # ---- persistent SBUF tensors ----
            zn = big.tile([128, N], f32, tag="zn")  # partition p: rows p*64+t
            znhat = big.tile([128, N], bf16, tag="znhat")  # normalized z, bf16
            zsb = big.tile([128, SLAB], bf16, tag="zsb")  # raw slab, bf16
            zhatT = big.tile([128, N], bf16, tag="zhatT")  # normalized z, transposed
            slabT = big.tile([128, SLAB], bf16, tag="slabT")  # raw slab, transposed
            zs_n = big.tile([128, SLAB], f32, tag="zs_n")
            zp_n = big.tile([128, SLAB], f32, tag="zp_n")
            eye_t = stats.tile([128, 128], f32, tag="eye")
            eye_b = stats.tile([128, 128], bf16, tag="eye_b")
            s_full = stats.tile([128, NT], f32, tag="s_full")  # row sumsq of z
            invn = stats.tile([128, NT], f32, tag="invn")  # 1/||z_r||
            s_s = stats.tile([128, ST], f32, tag="s_s")
            s_p = stats.tile([128, ST], f32, tag="s_p")
            sc2 = stats.tile([128, ST], f32, tag="sc2")  # 2/||z_slab_r||
            invn_p = stats.tile([128, ST], f32, tag="invn_p")
            posdot = stats.tile([128, ST], f32, tag="posdot")
            post1 = stats.tile([128, ST], f32, tag="post1")
            ra = stats.tile([128, NT], f32, tag="ra")  # rsqrt scratch
            rb = stats.tile([128, NT], f32, tag="rb")
            rh = stats.tile([128, NT], f32, tag="rh")
            rowparts = stats.tile([128, ST * NB], f32, tag="rowparts")
            rowsums = stats.tile([128, ST], f32, tag="rowsums")
            outbuf = stats.tile([128, 16], f32, tag="outbuf")
            waste = stats.tile([128, 2048], f32, tag="waste")  # exp values, unread
            sq_scr = stats.tile([128, 128], f32, tag="sq_scr")  # STT out, unread
            sq_scr2 = stats.tile([128, 128], f32, tag="sq_scr2")  # ACT square out
            # poly-ln scratch, all [128, ST]
            lx = stats.tile([128, ST], f32, tag="lx")
            lu = stats.tile([128, ST], u32, tag="lu")
            le = stats.tile([128, ST], f32, tag="le")
            lm = stats.tile([128, ST], u32, tag="lm")
            lnum = stats.tile([128, ST], f32, tag="lnum")
            lden = stats.tile([128, ST], f32, tag="lden")
            lt = stats.tile([128, ST], f32, tag="lt")
            lw = stats.tile([128, ST], f32, tag="lw")
            lp = stats.tile([128, ST], f32, tag="lp")

            def sumsq(a, b, acc):
                # acc[p] = sum_f a[p,f]*b[p,f]; out tile is scratch
                nc.vector.scalar_tensor_tensor(
                    sq_scr[:], a, 1.0, b, OP.mult, OP.mult, accum_out=acc
                )

            def sumsq_act(a, acc):
                nc.scalar.activation(
                    sq_scr2[:], a, AF.Square, bias=0.0, scale=1.0, accum_out=acc
                )

            def rsqrt(s_ap, out_ap, c):
                # out = 1/sqrt(s): quake seed + 2 Newton steps, all on DVE.
                # The MAGIC - (bits>>1) subtraction runs in f32 value domain
                # (uint add/sub wraparound is unreliable here); the ~2^-18
                # relative rounding this adds is irrelevant for a seed.
                bits = s_ap.bitcast(u32)
                sa = ra[:, 0:c]
                sb = rb[:, 0:c]
                sh = rh[:, 0:c]
                sa_u = sa.bitcast(u32)
                nc.vector.tensor_scalar(sa_u, bits, 1, None, OP.logical_shift_right)
                nc.vector.tensor_copy(sb, sa_u)  # u32 -> f32 value
                nc.vector.tensor_scalar(
                    sb, sb, float(MAGIC), -1.0, OP.subtract, OP.mult
                )  # MAGIC - v
                nc.vector.tensor_copy(sa_u, sb)  # f32 value -> u32 bits
                nc.vector.tensor_mul(sh, sa, sa)
                nc.vector.tensor_mul(sh, sh, s_ap)
                nc.vector.tensor_scalar(sh, sh, -0.5, 1.5, OP.mult, OP.add)
                nc.vector.tensor_mul(sb, sa, sh)
                nc.vector.tensor_mul(sh, sb, sb)
                nc.vector.tensor_mul(sh, sh, s_ap)
                nc.vector.tensor_scalar(sh, sh, -0.5, 1.5, OP.mult, OP.add)
                nc.vector.tensor_mul(out_ap, sb, sh)

            def rsqrt1(s_ap, out_ap, c):
                # single-Newton variant (rel err ~1.7e-3 -> ~-4e-4 bias; fine
                # for column scales feeding exp)
                bits = s_ap.bitcast(u32)
                sa = ra[:, 0:c]
                sb = rb[:, 0:c]
                sh = rh[:, 0:c]
                sa_u = sa.bitcast(u32)
                nc.vector.tensor_scalar(sa_u, bits, 1, None, OP.logical_shift_right)
                nc.vector.tensor_copy(sb, sa_u)
                nc.vector.tensor_scalar(
                    sb, sb, float(MAGIC), -1.0, OP.subtract, OP.mult
                )
                nc.vector.tensor_copy(sa_u, sb)
                nc.vector.tensor_mul(sh, sa, sa)
                nc.vector.tensor_mul(sh, sh, s_ap)
                nc.vector.tensor_scalar(sh, sh, -0.5, 1.5, OP.mult, OP.add)
                nc.vector.tensor_mul(out_ap, sa, sh)

            nc.sync.dma_start(eye_t[:], eye[:])
            nc.vector.tensor_copy(eye_b[:], eye_t[:])

            # ---- loads: per-partition contiguous (partition p <- rows p*K+i) ----
            # Order matters: the slab (zs) gates the whole main loop, then the
            # first two z chunks (first column block), then zp (positives).
            zv = z.rearrange("(p n) d -> p n d", p=128)  # [128, 64, 128]
            zsv = zs.rearrange("(p n) d -> p n d", p=128)
            zpv = zp.rearrange("(p n) d -> p n d", p=128)
            nc.sync.dma_start(zs_n[:], zsv[:])

            def load_chunk(g):
                nc.sync.dma_start(
                    zn[:, g * GT * 128 : (g + 1) * GT * 128],
                    zv[:, g * GT : (g + 1) * GT, :],
                )

            load_chunk(0)
            load_chunk(1)
            nc.sync.dma_start(zp_n[:], zpv[:])
            for g in range(2, GROUPS):
                load_chunk(g)

            # ---- slab: sumsq -> sc2 (needed by main exp), raw transpose ----
            for t in range(ST):
                zst = zs_n[:, t * 128 : (t + 1) * 128]
                sumsq(zst, zst, s_s[:, t : t + 1])
            rsqrt1(s_s[:], sc2[:], ST)
            nc.vector.tensor_scalar(sc2[:], sc2[:], 2.0, None, OP.mult)

            # slab scaled transpose -> slabT bf16: pre-scaling rows by
            # 2/||row|| here makes the PSUM logits fully scaled, so the exp
            # runs with a constant scale.
            for t in range(ST):
                nc.vector.tensor_scalar_mul(
                    zsb[:, t * 128 : (t + 1) * 128],
                    zs_n[:, t * 128 : (t + 1) * 128],
                    sc2[:, t : t + 1],
                )
            ppsb = mm_ps_pool.tile([128, 2048], f32, tag="mm")
            ppsb_b = ppsb[:, 0:1024].bitcast(bf16)[:, 0:1024]
            for t in range(ST):
                nc.tensor.transpose(
                    ppsb_b[:, t * 128 : (t + 1) * 128],
                    zsb[:, t * 128 : (t + 1) * 128],
                    eye_b[:],
                )
            nc.vector.tensor_copy(slabT[:], ppsb_b[:])

            # ---- full-z prep: sumsq -> invn -> bf16 cast -> diag matmul ----
            for g in range(GROUPS):
                lo, hi = g * GT, (g + 1) * GT
                for i in range(GT):
                    t = g * GT + i
                    znt = zn[:, t * 128 : (t + 1) * 128]
                    if t % 2 == 1 and g < 4:
                        sumsq_act(znt, s_full[:, t : t + 1])
                    else:
                        sumsq(znt, znt, s_full[:, t : t + 1])
                def scale_tiles(a, b):
                    for t in range(a, b):
                        nc.vector.tensor_scalar_mul(
                            znhat[:, t * 128 : (t + 1) * 128],
                            zn[:, t * 128 : (t + 1) * 128],
                            invn[:, t : t + 1],
                        )

                if g < 4:
                    rsqrt1(s_full[:, lo:hi], invn[:, lo:hi], GT)
                    scale_tiles(lo, hi)
                elif g == GROUPS - 1:
                    rsqrt1(
                        s_full[:, 4 * GT : NT], invn[:, 4 * GT : NT], NT - 4 * GT
                    )
                    scale_tiles(4 * GT, NT)
            # ---- transpose blocks + main loop, emission-interleaved so the
            # shared PSUM pool's in-order slot allocator never makes a main
            # tile wait on a far-future prep block (or vice versa) ----
            def prep_block(blk):
                pps = mm_ps_pool.tile([128, 2048], f32, tag="mm")
                ppsb16 = pps[:].bitcast(bf16)[:, 0:2048]
                for j in range(16):
                    t = blk * 16 + j
                    nc.tensor.transpose(
                        ppsb16[:, j * 128 : (j + 1) * 128],
                        znhat[:, t * 128 : (t + 1) * 128],
                        eye_b[:],
                    )
                if blk < 1:
                    nc.scalar.copy(zhatT[:, blk * 2048 : (blk + 1) * 2048], ppsb16)
                else:
                    nc.vector.tensor_copy(
                        zhatT[:, blk * 2048 : (blk + 1) * 2048], ppsb16
                    )

            def main_tile(nb, m):
                ps = mm_ps_pool.tile([128, 2048], f32, tag="mm")
                for h in range(4):
                    col = nb * 2048 + h * 512
                    nc.tensor.matmul(
                        ps[:, h * 512 : (h + 1) * 512],
                        lhsT=slabT[:, m * 128 : (m + 1) * 128],
                        rhs=zhatT[:, col : col + 512],
                        start=True,
                        stop=True,
                    )
                nc.scalar.activation(
                    waste[:],
                    ps[:],
                    AF.Exp,
                    bias=0.0,
                    scale=1.0,
                    accum_out=rowparts[:, m * NB + nb : m * NB + nb + 1],
                )

            prep_block(0)
            main_tile(0, 0)
            main_tile(0, 1)
            main_tile(0, 2)
            main_tile(0, 3)
            prep_block(1)
            main_tile(0, 4)
            main_tile(0, 5)
            main_tile(0, 6)
            main_tile(0, 7)
            prep_block(2)
            for m in range(4):
                main_tile(1, m)
            prep_block(3)
            for m in range(4, ST):
                main_tile(1, m)
            for nb in range(2, NB):
                for m in range(ST):
                    main_tile(nb, m)

            # ---- positives (off critical path) ----
            for t in range(ST):
                zst = zs_n[:, t * 128 : (t + 1) * 128]
                zpt = zp_n[:, t * 128 : (t + 1) * 128]
                sumsq(zpt, zpt, s_p[:, t : t + 1])
                sumsq(zst, zpt, posdot[:, t : t + 1])
            rsqrt1(s_p[:], invn_p[:], ST)
            # pos = posdot * (2*invn_s) * invn_p
            nc.vector.tensor_mul(post1[:], posdot[:], sc2[:])
            nc.vector.tensor_mul(outbuf[:, 8:16], post1[:], invn_p[:])

            # ---- epilogue: lse = log(rowsum - e^2) via exponent+poly ----
            nc.vector.tensor_reduce(
                rowsums[:],
                rowparts[:].rearrange("p (m n) -> p m n", m=ST),
                axis=mybir.AxisListType.X,
                op=OP.add,
            )
            nc.vector.tensor_scalar(lx[:], rowsums[:], EXP2, None, OP.subtract)
            bits = lx[:].bitcast(u32)
            # exponent (with IEEE bias): e = bits >> 23
            nc.vector.tensor_scalar(lu[:], bits, 23, None, OP.logical_shift_right)
            nc.vector.tensor_copy(le[:], lu[:])  # uint -> f32 convert
            # mantissa in [1, 2): m = (bits & 0x7fffff) | 0x3f800000
            nc.vector.tensor_scalar(
                lm[:], bits, 0x007FFFFF, 0x3F800000, OP.bitwise_and, OP.bitwise_or
            )
            mf = lm[:].bitcast(f32)
            # t = (m-1)/(m+1); ln(m) = 2t(1 + w/3 + w^2/5 + w^3/7 + w^4/9), w=t^2
            nc.vector.tensor_scalar(lnum[:], mf, 1.0, None, OP.subtract)
            nc.vector.tensor_scalar(lden[:], mf, 1.0, None, OP.add)
            nc.vector.reciprocal(lden[:], lden[:])
            nc.vector.tensor_mul(lt[:], lnum[:], lden[:])
            nc.vector.tensor_mul(lw[:], lt[:], lt[:])
            nc.vector.tensor_scalar(lp[:], lw[:], 2.0 / 9.0, 2.0 / 7.0, OP.mult, OP.add)
            nc.vector.tensor_mul(lp[:], lp[:], lw[:])
            nc.vector.tensor_scalar(lp[:], lp[:], 2.0 / 5.0, None, OP.add)
            nc.vector.tensor_mul(lp[:], lp[:], lw[:])
            nc.vector.tensor_scalar(lp[:], lp[:], 2.0 / 3.0, None, OP.add)
            nc.vector.tensor_mul(lp[:], lp[:], lw[:])
            nc.vector.tensor_scalar(lp[:], lp[:], 2.0, None, OP.add)
            nc.vector.tensor_mul(lp[:], lp[:], lt[:])  # 2t(1+w/3+...) = ln(m)
            # lse = (e - 127)*ln2 + ln(m)
            nc.vector.tensor_scalar(le[:], le[:], 127.0, None, OP.subtract)
            nc.vector.scalar_tensor_tensor(
                outbuf[:, 0:8], le[:], LN2, lp[:], OP.mult, OP.add
            )
            nc.sync.dma_start(out[:], outbuf[:])

    nc.compile()
    return nc


_NC_CACHE = {}


def _get_nc():
    if "nc" not in _NC_CACHE:
        _NC_CACHE["nc"] = build_nc()
    return _NC_CACHE["nc"]


def kernel(z_i, z_j):
    z_i = np.asarray(z_i, dtype=np.float32)
    z_j = np.asarray(z_j, dtype=np.float32)
    z = np.ascontiguousarray(np.concatenate([z_i, z_j], axis=0))
    eye = np.eye(128, dtype=np.float32)
    in_maps = []
    for c in range(CORES):
        r0 = c * SLAB
        p0 = (r0 + B) % N
        in_maps.append(
            {
                "z": z,
                "zs": np.ascontiguousarray(z[r0 : r0 + SLAB]),
                "zp": np.ascontiguousarray(z[p0 : p0 + SLAB]),
                "eye": eye,
            }
        )
    nc = _get_nc()
    kwargs = {}
    tdir = os.environ.get("NTX_TRACE_DIR")
    if tdir:
        kwargs = {"trace": True, "tmpdir": tdir, "trace_cores": [0]}
    res = run_bass_kernel_spmd(nc, in_maps, core_ids=list(range(CORES)), **kwargs)
    if tdir:
        _NC_CACHE["last_results"] = res
    tot = 0.0
    for c in range(CORES):
        o = res.results[c]["out"].astype(np.float64)
        tot += o[:, 0:8].sum() - o[:, 8:16].sum()
    return np.float32(tot / N)
